# revision 1
# baseline (speedup 1.0000x reference)
"""Trainium2 Bass kernel for ClassForgeEnsembleGNN (SAGE -> GAT -> RGCN ensemble).

Strategy (8 NeuronCores, SPMD):
  - Partition nodes into 8 contiguous shards (6250 nodes each); each core owns
    the edges whose *target* lands in its shard (scatter destinations local).
  - Node features for gathers are replicated (x) or all-gathered between
    stages (x1, x2) via device collectives.
  - Scatter-aggregation is done with the "selection matrix" matmul trick:
    for a tile of 128 edges targeting a 128-node block, build
    S[e, n] = (dst_local[e] == n) * w[e] on the vector engine (iota compare),
    then one TensorE matmul accumulates messages into the block PSUM.
  - Per-edge gathers use GPSIMD indirect DMA (row gather by src index).
  - GAT attention: logits a_s/a_d are precomputed per node in the SAGE
    epilogue (attention vectors folded into the weights: a_s = x1 @ (W_gat@att)),
    softmax runs without max-subtraction (logits are O(1); validated), and
    W_gat is applied after aggregation (linearity).
  - RGCN: per-(relation, block) edge groups; 1/cnt_r weights folded into S;
    W_rgcn[r] applied after aggregation (linearity).

The Bass program is built per invocation (edge structure is baked in as
metadata tensors + per-block tile counts; counts are made uniform across the
8 cores by padding with inert edges: dst_local=-1 never matches the iota).
"""

import sys
import os

for _p in ("/opt/trn_rl_repo", "/root/.axon_site/_ro/trn_rl_repo"):
    if os.path.isdir(_p) and _p not in sys.path:
        sys.path.append(_p)

import numpy as np

import concourse.bacc as bacc
import concourse.bass as bass
import concourse.mybir as mybir
import concourse.tile as tile
from concourse.bass_utils import run_bass_kernel_spmd
from concourse.masks import make_identity

P = 128
NCORES = 8
N = 50000
E = 400000
D = 128
H = 2
R = 5
NEG = 0.2
SH = N // NCORES            # 6250 nodes per shard
B = (SH + P - 1) // P       # 49 blocks per shard (last block has 106 valid)

f32 = mybir.dt.float32
i32 = mybir.dt.int32
AF = mybir.ActivationFunctionType
ALU = mybir.AluOpType


def _pack_groups(groups_per_core, payloads_per_core, pads, dtypes, n_groups):
    """Pack per-edge payloads into fixed tile slots, grouped by `group` id.

    Returns (caps[g] tiles per group, offs[g] tile offsets, T total tiles,
    per-core dict of [128, T] arrays).  Group capacities are uniform across
    cores (max over cores), padded entries get `pads` values.
    """
    counts = np.stack(
        [np.bincount(g, minlength=n_groups) for g in groups_per_core]
    )  # [K, G]
    caps = ((counts + P - 1) // P).max(0)  # [G]
    offs = np.zeros(n_groups + 1, np.int64)
    np.cumsum(caps, out=offs[1:])
    T = int(offs[-1])
    out = []
    for k, g in enumerate(groups_per_core):
        order = np.argsort(g, kind="stable")
        gs = g[order]
        starts = np.zeros(n_groups, np.int64)
        np.cumsum(counts[k], out=starts[1:] if False else None)  # placeholder
        starts = np.concatenate([[0], np.cumsum(counts[k])])[:-1]
        rank = np.arange(len(gs)) - starts[gs]
        slot = offs[gs] * P + rank
        core_out = {}
        for name in payloads_per_core[k]:
            a = np.full(T * P, pads[name], dtype=dtypes[name])
            a[slot] = payloads_per_core[k][name][order]
            core_out[name] = np.ascontiguousarray(a.reshape(T, P).T)
        out.append(core_out)
    return caps, offs[:-1], T, out


def _preprocess(x, edge_index, edge_type):
    src = edge_index[0].astype(np.int64)
    dst = edge_index[1].astype(np.int64)
    et = edge_type.astype(np.int64)

    cnt = np.bincount(dst, minlength=N).astype(np.float32)
    w_sage_all = 1.0 / np.maximum(cnt, 1.0)

    cnt_r = np.zeros((R, N), np.float32)
    for r in range(R):
        cnt_r[r] = np.bincount(dst[et == r], minlength=N)
    w_r_all = 1.0 / np.maximum(cnt_r, 1.0)

    shard_of = dst // SH
    sage_groups, sage_pay = [], []
    gat_groups, gat_pay = [], []
    rgcn_groups, rgcn_pay = [], []
    loop_ids = np.arange(SH, dtype=np.int64)
    for k in range(NCORES):
        sel = shard_of == k
        es, ed, er = src[sel], dst[sel], et[sel]
        dloc = ed - k * SH
        blk = dloc // P
        dst_local = (dloc % P).astype(np.float32)
        # SAGE
        sage_groups.append(blk)
        sage_pay.append(
            {"src": es.astype(np.int32), "dl": dst_local, "w": w_sage_all[ed]}
        )
        # GAT: edges + self loops of shard nodes
        gl = loop_ids + k * SH
        g_src = np.concatenate([es, gl])
        g_dst = np.concatenate([ed, gl])
        g_dloc = g_dst - k * SH
        gat_groups.append(g_dloc // P)
        gat_pay.append(
            {
                "src": g_src.astype(np.int32),
                "di": g_dst.astype(np.int32),
                "dl": (g_dloc % P).astype(np.float32),
            }
        )
        # RGCN: group id r*B + blk
        rgcn_groups.append(er * B + blk)
        rgcn_pay.append(
            {"src": es.astype(np.int32), "dl": dst_local, "w": w_r_all[er, ed]}
        )

    caps_s, offs_s, T_s, meta_s = _pack_groups(
        sage_groups, sage_pay,
        {"src": 0, "dl": -1.0, "w": 0.0},
        {"src": np.int32, "dl": np.float32, "w": np.float32}, B)
    caps_g, offs_g, T_g, meta_g = _pack_groups(
        gat_groups, gat_pay,
        {"src": 0, "di": 0, "dl": -1.0},
        {"src": np.int32, "di": np.int32, "dl": np.float32}, B)
    caps_r, offs_r, T_r, meta_r = _pack_groups(
        rgcn_groups, rgcn_pay,
        {"src": 0, "dl": -1.0, "w": 0.0},
        {"src": np.int32, "dl": np.float32, "w": np.float32}, R * B)

    return dict(
        caps_s=caps_s, offs_s=offs_s, T_s=T_s, meta_s=meta_s,
        caps_g=caps_g, offs_g=offs_g, T_g=T_g, meta_g=meta_g,
        caps_r=caps_r, offs_r=offs_r, T_r=T_r, meta_r=meta_r,
    )


def _build_program(pp):
    caps_s, offs_s, T_s = pp["caps_s"], pp["offs_s"], pp["T_s"]
    caps_g, offs_g, T_g = pp["caps_g"], pp["offs_g"], pp["T_g"]
    caps_r, offs_r, T_r = pp["caps_r"], pp["offs_r"], pp["T_r"]

    nc = bacc.Bacc("TRN2", target_bir_lowering=False, debug=False,
                   num_devices=NCORES)

    x_dram = nc.dram_tensor("x", [N, D], f32, kind="ExternalInput")
    xt_dram = nc.dram_tensor("xt", [B, P, P], f32, kind="ExternalInput")
    msi = nc.dram_tensor("msi", [P, T_s], i32, kind="ExternalInput")
    msf = nc.dram_tensor("msf", [P, 2 * T_s], f32, kind="ExternalInput")
    mgi = nc.dram_tensor("mgi", [P, T_g], i32, kind="ExternalInput")
    mgd = nc.dram_tensor("mgd", [P, T_g], i32, kind="ExternalInput")
    mgf = nc.dram_tensor("mgf", [P, T_g], f32, kind="ExternalInput")
    mri = nc.dram_tensor("mri", [P, T_r], i32, kind="ExternalInput")
    mrf = nc.dram_tensor("mrf", [P, 2 * T_r], f32, kind="ExternalInput")
    wsl = nc.dram_tensor("wsl", [D, D], f32, kind="ExternalInput")
    wsr = nc.dram_tensor("wsr", [D, D], f32, kind="ExternalInput")
    bs = nc.dram_tensor("bs", [P, 1], f32, kind="ExternalInput")
    vsd = nc.dram_tensor("vsd", [D, 4], f32, kind="ExternalInput")
    wg0 = nc.dram_tensor("wg0", [D, D], f32, kind="ExternalInput")
    wg1 = nc.dram_tensor("wg1", [D, D], f32, kind="ExternalInput")
    bg = nc.dram_tensor("bg", [P, 1], f32, kind="ExternalInput")
    wroot = nc.dram_tensor("wroot", [D, D], f32, kind="ExternalInput")
    wr = nc.dram_tensor("wr", [R, D, D], f32, kind="ExternalInput")
    br = nc.dram_tensor("br", [P, 1], f32, kind="ExternalInput")
    out_dram = nc.dram_tensor("out", [SH, 3 * D], f32, kind="ExternalOutput")

    rg = [list(range(NCORES))]

    with tile.TileContext(nc) as tc:
        with (
            tc.tile_pool(name="const", bufs=1) as cb,
            tc.tile_pool(name="sbuf", bufs=4) as sb,
            tc.tile_pool(name="psum", bufs=2, space="PSUM") as ps,
            tc.tile_pool(name="dram", bufs=1, space="DRAM") as dr,
        ):
            ident = cb.tile([P, P], f32)
            make_identity(nc, ident[:])
            iota_i = cb.tile([P, P], i32)
            nc.gpsimd.iota(iota_i[:], pattern=[[1, P]], base=0,
                           channel_multiplier=0)
            iota_f = cb.tile([P, P], f32)
            nc.vector.tensor_copy(iota_f[:], iota_i[:])

            def load_const(name, dram, shape):
                t = cb.tile(shape, f32, name=name)
                nc.sync.dma_start(t[:], dram[:])
                return t

            wsl_sb = load_const("wsl_sb", wsl, [D, D])
            wsr_sb = load_const("wsr_sb", wsr, [D, D])
            bs_sb = load_const("bs_sb", bs, [P, 1])
            vsd_sb = load_const("vsd_sb", vsd, [D, 4])
            wg0_sb = load_const("wg0_sb", wg0, [D, D])
            wg1_sb = load_const("wg1_sb", wg1, [D, D])
            bg_sb = load_const("bg_sb", bg, [P, 1])
            wroot_sb = load_const("wroot_sb", wroot, [D, D])
            br_sb = load_const("br_sb", br, [P, 1])
            wr_sb = cb.tile([P, R * D], f32)
            for r in range(R):
                nc.sync.dma_start(wr_sb[:, r * D:(r + 1) * D], wr[r, :, :])

            msi_sb = cb.tile([P, T_s], i32)
            nc.sync.dma_start(msi_sb[:], msi[:])
            msf_sb = cb.tile([P, 2 * T_s], f32)
            nc.sync.dma_start(msf_sb[:], msf[:])
            mgi_sb = cb.tile([P, T_g], i32)
            nc.sync.dma_start(mgi_sb[:], mgi[:])
            mgd_sb = cb.tile([P, T_g], i32)
            nc.sync.dma_start(mgd_sb[:], mgd[:])
            mgf_sb = cb.tile([P, T_g], f32)
            nc.sync.dma_start(mgf_sb[:], mgf[:])
            mri_sb = cb.tile([P, T_r], i32)
            nc.sync.dma_start(mri_sb[:], mri[:])
            mrf_sb = cb.tile([P, 2 * T_r], f32)
            nc.sync.dma_start(mrf_sb[:], mrf[:])

            x2T_sb = cb.tile([P, B * P], f32)
            x3acc = cb.tile([P, B * P], f32)

            cc1_in = dr.tile([SH, 132], f32)
            cc1_out = dr.tile([N, 132], f32, addr_space="Shared")
            ad_in = dr.tile([SH, 2], f32)
            ad_out = dr.tile([N, 2], f32, addr_space="Shared")
            cc2_in = dr.tile([SH, D], f32)
            cc2_out = dr.tile([N, D], f32, addr_space="Shared")

            # ================= Stage 1: SAGE =================
            for b in range(B):
                vld = min(P, SH - b * P)
                r0 = b * P
                cap = int(caps_s[b])
                off = int(offs_s[b])
                aggT = sb.tile([P, P], f32, tag="aggT")
                if cap > 0:
                    pa = ps.tile([P, P], f32, tag="acc")
                    for j in range(cap):
                        t = off + j
                        xg = sb.tile([P, P], f32, tag="xg")
                        nc.gpsimd.indirect_dma_start(
                            out=xg[:], out_offset=None, in_=x_dram[:],
                            in_offset=bass.IndirectOffsetOnAxis(
                                ap=msi_sb[:, t:t + 1], axis=0))
                        S = sb.tile([P, P], f32, tag="S")
                        nc.vector.tensor_scalar(
                            out=S[:], in0=iota_f[:],
                            scalar1=msf_sb[:, 2 * t:2 * t + 1],
                            scalar2=msf_sb[:, 2 * t + 1:2 * t + 2],
                            op0=ALU.is_equal, op1=ALU.mult)
                        nc.tensor.matmul(pa[:], lhsT=xg[:], rhs=S[:],
                                         start=(j == 0), stop=(j == cap - 1))
                    nc.scalar.copy(aggT[:], pa[:])
                else:
                    nc.vector.memset(aggT[:], 0.0)
                xT = sb.tile([P, P], f32, tag="xT")
                nc.sync.dma_start(xT[:], xt_dram[b, :, :])
                pm = ps.tile([P, P], f32, tag="mm")
                nc.tensor.matmul(pm[:], lhsT=wsl_sb[:], rhs=aggT[:],
                                 start=True, stop=False)
                nc.tensor.matmul(pm[:], lhsT=wsr_sb[:], rhs=xT[:],
                                 start=False, stop=True)
                x1T = sb.tile([P, P], f32, tag="x1T")
                nc.scalar.activation(x1T[:], pm[:], AF.Relu,
                                     bias=bs_sb[:, 0:1], scale=1.0)
                pasd = ps.tile([P, 4], f32, tag="asd")
                nc.tensor.matmul(pasd[:], lhsT=x1T[:], rhs=vsd_sb[:],
                                 start=True, stop=True)
                asd = sb.tile([P, 4], f32, tag="asdsb")
                nc.scalar.copy(asd[:], pasd[:])
                ptr = ps.tile([P, P], f32, tag="tr")
                nc.tensor.transpose(ptr[:], x1T[:], ident[:])
                x1n = sb.tile([P, P], f32, tag="x1n")
                nc.scalar.copy(x1n[:], ptr[:])
                nc.sync.dma_start(cc1_in[r0:r0 + vld, 0:D], x1n[:vld, :])
                nc.sync.dma_start(cc1_in[r0:r0 + vld, D:D + 2], asd[:vld, 0:2])
                nc.sync.dma_start(ad_in[r0:r0 + vld, :], asd[:vld, 2:4])
                nc.sync.dma_start(out_dram[r0:r0 + vld, 0:D], x1n[:vld, :])

            nc.gpsimd.collective_compute(
                "AllGather", ALU.bypass, replica_groups=rg,
                ins=[cc1_in[:]], outs=[cc1_out[:]])
            nc.gpsimd.collective_compute(
                "AllGather", ALU.bypass, replica_groups=rg,
                ins=[ad_in[:]], outs=[ad_out[:]])

            # ================= Stage 2: GAT =================
            for b in range(B):
                vld = min(P, SH - b * P)
                r0 = b * P
                cap = int(caps_g[b])
                off = int(offs_g[b])
                pg = ps.tile([P, 258], f32, tag="acc")
                for j in range(cap):
                    t = off + j
                    xg1 = sb.tile([P, 132], f32, tag="xg")
                    nc.gpsimd.indirect_dma_start(
                        out=xg1[:], out_offset=None, in_=cc1_out[:],
                        in_offset=bass.IndirectOffsetOnAxis(
                            ap=mgi_sb[:, t:t + 1], axis=0))
                    adg = sb.tile([P, 2], f32, tag="adg")
                    nc.gpsimd.indirect_dma_start(
                        out=adg[:], out_offset=None, in_=ad_out[:],
                        in_offset=bass.IndirectOffsetOnAxis(
                            ap=mgd_sb[:, t:t + 1], axis=0))
                    rhs = sb.tile([P, 258], f32, tag="grhs")
                    ev = sb.tile([P, 2], f32, tag="ev")
                    nc.vector.tensor_add(ev[:], xg1[:, D:D + 2], adg[:])
                    ev2 = sb.tile([P, 2], f32, tag="ev2")
                    nc.vector.tensor_scalar(out=ev2[:], in0=ev[:],
                                            scalar1=NEG, scalar2=None,
                                            op0=ALU.mult)
                    ev3 = sb.tile([P, 2], f32, tag="ev3")
                    nc.vector.tensor_tensor(out=ev3[:], in0=ev2[:], in1=ev[:],
                                            op=ALU.max)
                    nc.scalar.activation(rhs[:, 256:258], ev3[:], AF.Exp)
                    nc.scalar.activation(rhs[:, 0:D], xg1[:, 0:D], AF.Copy,
                                         scale=rhs[:, 256:257])
                    nc.scalar.activation(rhs[:, D:2 * D], xg1[:, 0:D], AF.Copy,
                                         scale=rhs[:, 257:258])
                    S = sb.tile([P, P], f32, tag="S")
                    nc.vector.tensor_scalar(
                        out=S[:], in0=iota_f[:],
                        scalar1=mgf_sb[:, t:t + 1], scalar2=None,
                        op0=ALU.is_equal)
                    nc.tensor.matmul(pg[:], lhsT=S[:], rhs=rhs[:],
                                     start=(j == 0), stop=(j == cap - 1))
                denr = sb.tile([P, 2], f32, tag="denr")
                nc.vector.reciprocal(denr[:], pg[:, 256:258])
                nm0 = sb.tile([P, P], f32, tag="nm0")
                nc.vector.tensor_scalar(out=nm0[:], in0=pg[:, 0:D],
                                        scalar1=denr[:, 0:1], scalar2=None,
                                        op0=ALU.mult)
                nm1 = sb.tile([P, P], f32, tag="nm1")
                nc.vector.tensor_scalar(out=nm1[:], in0=pg[:, D:2 * D],
                                        scalar1=denr[:, 1:2], scalar2=None,
                                        op0=ALU.mult)
                pt0 = ps.tile([P, P], f32, tag="tr")
                nc.tensor.transpose(pt0[:], nm0[:], ident[:])
                t0 = sb.tile([P, P], f32, tag="t0")
                nc.scalar.copy(t0[:], pt0[:])
                pt1 = ps.tile([P, P], f32, tag="tr")
                nc.tensor.transpose(pt1[:], nm1[:], ident[:])
                t1 = sb.tile([P, P], f32, tag="t1")
                nc.scalar.copy(t1[:], pt1[:])
                px2 = ps.tile([P, P], f32, tag="mm")
                nc.tensor.matmul(px2[:], lhsT=wg0_sb[:], rhs=t0[:],
                                 start=True, stop=False)
                nc.tensor.matmul(px2[:], lhsT=wg1_sb[:], rhs=t1[:],
                                 start=False, stop=True)
                x2T_blk = x2T_sb[:, r0:r0 + P]
                nc.scalar.activation(x2T_blk, px2[:], AF.Relu,
                                     bias=bg_sb[:, 0:1], scale=0.5)
                px3 = ps.tile([P, P], f32, tag="mm")
                nc.tensor.matmul(px3[:], lhsT=wroot_sb[:], rhs=x2T_blk,
                                 start=True, stop=True)
                nc.scalar.activation(x3acc[:, r0:r0 + P], px3[:], AF.Identity,
                                     bias=br_sb[:, 0:1])
                ptr2 = ps.tile([P, P], f32, tag="tr")
                nc.tensor.transpose(ptr2[:], x2T_blk, ident[:])
                x2n = sb.tile([P, P], f32, tag="x2n")
                nc.scalar.copy(x2n[:], ptr2[:])
                nc.sync.dma_start(cc2_in[r0:r0 + vld, :], x2n[:vld, :])
                nc.sync.dma_start(out_dram[r0:r0 + vld, D:2 * D],
                                  x2n[:vld, :])

            nc.gpsimd.collective_compute(
                "AllGather", ALU.bypass, replica_groups=rg,
                ins=[cc2_in[:]], outs=[cc2_out[:]])

            # ================= Stage 3: RGCN =================
            for r in range(R):
                for b in range(B):
                    cap = int(caps_r[r * B + b])
                    if cap == 0:
                        continue
                    off = int(offs_r[r * B + b])
                    r0 = b * P
                    pa = ps.tile([P, P], f32, tag="acc")
                    for j in range(cap):
                        t = off + j
                        xg2 = sb.tile([P, P], f32, tag="xg")
                        nc.gpsimd.indirect_dma_start(
                            out=xg2[:], out_offset=None, in_=cc2_out[:],
                            in_offset=bass.IndirectOffsetOnAxis(
                                ap=mri_sb[:, t:t + 1], axis=0))
                        S = sb.tile([P, P], f32, tag="S")
                        nc.vector.tensor_scalar(
                            out=S[:], in0=iota_f[:],
                            scalar1=mrf_sb[:, 2 * t:2 * t + 1],
                            scalar2=mrf_sb[:, 2 * t + 1:2 * t + 2],
                            op0=ALU.is_equal, op1=ALU.mult)
                        nc.tensor.matmul(pa[:], lhsT=xg2[:], rhs=S[:],
                                         start=(j == 0), stop=(j == cap - 1))
                    arT = sb.tile([P, P], f32, tag="aggT")
                    nc.scalar.copy(arT[:], pa[:])
                    pw = ps.tile([P, P], f32, tag="mm")
                    nc.tensor.matmul(pw[:], lhsT=wr_sb[:, r * D:(r + 1) * D],
                                     rhs=arT[:], start=True, stop=True)
                    nc.vector.tensor_add(x3acc[:, r0:r0 + P],
                                         x3acc[:, r0:r0 + P], pw[:])

            # ================= x3 output =================
            for b in range(B):
                vld = min(P, SH - b * P)
                r0 = b * P
                ptr3 = ps.tile([P, P], f32, tag="tr")
                nc.tensor.transpose(ptr3[:], x3acc[:, r0:r0 + P], ident[:])
                x3n = sb.tile([P, P], f32, tag="x3n")
                nc.scalar.copy(x3n[:], ptr3[:])
                nc.sync.dma_start(out_dram[r0:r0 + vld, 2 * D:3 * D],
                                  x3n[:vld, :])

    nc.compile()
    return nc


def kernel(x, edge_index, edge_type, W_sage_l, b_sage, W_sage_r,
           W_gat, att_src, att_dst, b_gat, W_rgcn, W_root, b_rgcn,
           _trace=False):
    x = np.asarray(x, np.float32)
    edge_index = np.asarray(edge_index)
    edge_type = np.asarray(edge_type)

    pp = _preprocess(x, edge_index, edge_type)
    nc = _build_program(pp)

    # attention vectors folded into GAT weights: a_s = x1 @ (W_gat[:,h,:] @ att_src[h])
    W_gat = np.asarray(W_gat, np.float32)
    v = np.empty((D, 4), np.float32)
    for h in range(H):
        v[:, h] = W_gat[:, h, :] @ np.asarray(att_src, np.float32)[h]
        v[:, 2 + h] = W_gat[:, h, :] @ np.asarray(att_dst, np.float32)[h]

    common = {
        "x": x,
        "wsl": np.asarray(W_sage_l, np.float32),
        "wsr": np.asarray(W_sage_r, np.float32),
        "bs": np.asarray(b_sage, np.float32).reshape(P, 1),
        "vsd": v,
        "wg0": W_gat[:, 0, :].copy(),
        "wg1": W_gat[:, 1, :].copy(),
        "bg": np.asarray(b_gat, np.float32).reshape(P, 1),
        "wroot": np.asarray(W_root, np.float32),
        "wr": np.asarray(W_rgcn, np.float32),
        "br": np.asarray(b_rgcn, np.float32).reshape(P, 1),
    }

    in_maps = []
    for k in range(NCORES):
        xs = np.zeros((B * P, D), np.float32)
        xs[:SH] = x[k * SH:(k + 1) * SH]
        xt = np.ascontiguousarray(
            xs.reshape(B, P, D).transpose(0, 2, 1))
        m = dict(common)
        m["xt"] = xt
        ms = pp["meta_s"][k]
        m["msi"] = ms["src"]
        msf = np.empty((P, 2 * pp["T_s"]), np.float32)
        msf[:, 0::2] = ms["dl"]
        msf[:, 1::2] = ms["w"]
        m["msf"] = msf
        mg = pp["meta_g"][k]
        m["mgi"] = mg["src"]
        m["mgd"] = mg["di"]
        m["mgf"] = mg["dl"]
        mr = pp["meta_r"][k]
        m["mri"] = mr["src"]
        mrf = np.empty((P, 2 * pp["T_r"]), np.float32)
        mrf[:, 0::2] = mr["dl"]
        mrf[:, 1::2] = mr["w"]
        m["mrf"] = mrf
        in_maps.append(m)

    res = run_bass_kernel_spmd(nc, in_maps, core_ids=list(range(NCORES)),
                               trace=_trace)
    out = np.concatenate([res.results[k]["out"] for k in range(NCORES)], 0)
    if _trace:
        return out, res
    return out
